# revision 7
# baseline (speedup 1.0000x reference)
"""DLSE (resettable elementwise log-sum-exp scan) Trainium2 kernel.

Strategy (8 NeuronCores, sequence-parallel over time):
  - Each core owns a 256-step time chunk (2 blocks of 128 steps, block = partition dim).
  - Layout on device: [t (partition), b*128 + a (free)] for the [H,H] state tensors.
  - gamma == identity (verified at runtime; general case falls back to a host
    implementation), so the scan monoid is an elementwise segmented
    log-add-exp: S[t] = log( sum_{j in segment(t)} exp(kv[j]) [+ exp(h0)] ).
    We compute A[t] = segmented prefix-sum of E[j] = exp(kv[j]) and S = ln(A).
  - The segmented prefix sum over a 128-block is a matmul with a host-built
    "segmented triangular" 0/1 matrix on the tensor engine.
  - Cross-core carries are exchanged with a single AllGather per layer of each
    core's local chunk-final state F; each core combines the gathered F rows
    with host-built 0/1 coefficients and injects the result via a K=1 matmul
    accumulated into the same PSUM groups as the prefix-sum matmuls.
"""

import sys
import numpy as np

sys.path.insert(0, "/opt/trn_rl_repo")

T, IN, H, OUTD, L = 2048, 128, 128, 128, 2
NCORE = 8
TC = T // NCORE      # 256 timesteps per core
NB = 2               # blocks per core
TB = 128             # timesteps per block
FREE = H * H         # 16384
CH = 512             # psum chunk columns for the cumsum matmuls
NCH = FREE // CH     # 32
GST = 1024           # G staging row chunk
QW = FREE // 8       # 2048, sub-block width for z / norm passes
NQ = FREE // QW      # 8

# matmul dtype for the big prefix-sum / carry matmuls:
#   "f32"  : exact fp32 (4 cycles/row on PE)
#   "f32r" : float32r (full rate, relaxed precision on HW)
MM_MODE = "f32"

_CACHE = {}


def _build_program(mm_mode):
    import concourse.bacc as bacc
    import concourse.tile as tile
    from concourse import mybir

    F32 = mybir.dt.float32
    F32R = mybir.dt.float32r
    AF = mybir.ActivationFunctionType
    AL = mybir.AluOpType
    AX = mybir.AxisListType

    def mmcast(ap):
        return ap.bitcast(F32R) if mm_mode == "f32r" else ap

    nc = bacc.Bacc("TRN2", target_bir_lowering=False, debug=False,
                   num_devices=NCORE)

    din = {}

    def dram_in(name, shape):
        din[name] = nc.dram_tensor(name, list(shape), F32,
                                   kind="ExternalInput").ap()

    dram_in("xT", (128, TC))
    dram_in("winT", (128, 128))
    dram_in("woutT", (128, 128))
    for nm in ("kT", "qT", "vT", "wzT", "weT"):
        dram_in(nm, (L, 128, 128))
    dram_in("bin", (128, 1))
    dram_in("bout", (128, 1))
    dram_in("bff", (L, 128, 1))
    dram_in("tri", (NB, 128, 128))
    dram_in("cross", (128, 128))
    dram_in("wrow", (1, NB * 128))
    dram_in("sufcol", (128, NB))
    dram_in("lnbias", (128, NB))
    dram_in("gam", (128, NCORE))
    dram_in("ident", (128, 128))

    out_d = nc.dram_tensor("outc", [TC, OUTD], F32, kind="ExternalOutput").ap()
    hfin_d = nc.dram_tensor("hfin", [L, FREE], F32, kind="ExternalOutput").ap()

    with tile.TileContext(nc) as tc:
        with (
            tc.tile_pool(name="wp", bufs=1) as wp,
            tc.tile_pool(name="big", bufs=1) as bigp,
            tc.tile_pool(name="sm", bufs=1) as smp,
            tc.tile_pool(name="scr", bufs=2) as scrp,
            tc.tile_pool(name="gs", bufs=2) as gsp,
            tc.tile_pool(name="psC", bufs=3, space="PSUM") as psCp,
            tc.tile_pool(name="psF", bufs=2, space="PSUM") as psFp,
            tc.tile_pool(name="psG", bufs=1, space="PSUM") as psGp,
            tc.tile_pool(name="dr", bufs=2, space="DRAM") as drp,
        ):
            # ---------- load persistent inputs ----------
            def load2(name, shape, src_ap):
                t = wp.tile(list(shape), F32, tag=name, name=name)
                nc.sync.dma_start(t[:], src_ap)
                return t

            winT = load2("winT", (128, 128), din["winT"][:, :])
            woutT = load2("woutT", (128, 128), din["woutT"][:, :])
            identt = load2("ident", (128, 128), din["ident"][:, :])
            kTt, qTt, vTt, wzTt, weTt = [], [], [], [], []
            for l in range(L):
                kTt.append(load2(f"kT{l}", (128, 128), din["kT"][l:l + 1, :, :]))
                qTt.append(load2(f"qT{l}", (128, 128), din["qT"][l:l + 1, :, :]))
                vTt.append(load2(f"vT{l}", (128, 128), din["vT"][l:l + 1, :, :]))
                wzTt.append(load2(f"wzT{l}", (128, 128), din["wzT"][l:l + 1, :, :]))
                weTt.append(load2(f"weT{l}", (128, 128), din["weT"][l:l + 1, :, :]))
            bint = load2("bin", (128, 1), din["bin"][:, :])
            boutt = load2("bout", (128, 1), din["bout"][:, :])
            bfft = load2("bff", (128, L), din["bff"][:, :, 0].transpose([1, 0]))
            trit = [load2(f"tri{i}", (128, 128), din["tri"][i:i + 1, :, :])
                    for i in range(NB)]
            crosst = load2("cross", (128, 128), din["cross"][:, :])
            wrowt = load2("wrow", (1, NB * 128), din["wrow"][:, :])
            sufct = load2("sufcol", (128, NB), din["sufcol"][:, :])
            lnbt = load2("lnbias", (128, NB), din["lnbias"][:, :])
            gamt = load2("gam", (128, NCORE), din["gam"][:, :])
            xTt = load2("xT", (128, TC), din["xT"][:, :])

            # ---------- persistent working tiles ----------
            E = [bigp.tile([128, FREE], F32, tag=f"E{i}", name=f"E{i}") for i in range(NB)]
            ktile = smp.tile([128, TC], F32, tag="k", name="k")
            qtile = smp.tile([128, TC], F32, tag="q", name="q")
            vtile = smp.tile([128, TC], F32, tag="v", name="v")
            embT = smp.tile([128, TC], F32, tag="embT", name="embT")
            lin1T = smp.tile([128, TC], F32, tag="lin1T", name="lin1T")
            lin2T = smp.tile([128, TC], F32, tag="lin2T", name="lin2T")
            zraw = smp.tile([128, TC], F32, tag="zraw", name="zraw")
            zT = smp.tile([128, TC], F32, tag="zT", name="zT")
            nsq = smp.tile([128, NQ * NB], F32, tag="nsq", name="nsq")
            n2 = smp.tile([128, NB], F32, tag="n2", name="n2")
            rn = smp.tile([128, NB], F32, tag="rn", name="rn")
            Fg = smp.tile([128, NCORE * 128], F32, tag="Fg", name="Fg")
            Gt = smp.tile([128, 128], F32, tag="Gt", name="Gt")
            outT = smp.tile([128, TC], F32, tag="outT", name="outT")
            outt = smp.tile([128, TC], F32, tag="outt", name="outt")

            # ---------- embT = W_in @ x.T + b_in ----------
            pg = psGp.tile([128, TC], F32, tag="psG", name="psG")
            nc.tensor.matmul(pg[:], winT[:], xTt[:], start=True, stop=True)
            nc.scalar.activation(embT[:], pg[:], AF.Identity, bias=bint[:, 0:1])

            lin_in = [embT, lin1T]
            lin_out = [lin1T, lin2T]

            for l in range(L):
                linT = lin_in[l]
                # ---------- k, q, v GEMMs: out [t, h] per block ----------
                for i in range(NB):
                    sl = slice(i * TB, (i + 1) * TB)
                    for (dst, wmat) in ((ktile, kTt[l]), (qtile, qTt[l]),
                                        (vtile, vTt[l])):
                        p = psGp.tile([128, TC], F32, tag="psG", name="psG")
                        nc.tensor.matmul(p[:, 0:TB], linT[:, sl], wmat[:],
                                         start=True, stop=True)
                        nc.scalar.copy(dst[:, sl], p[:, 0:TB])

                # ---------- E = exp(outer(k, v)) per block ----------
                for i in range(NB):
                    sl = slice(i * TB, (i + 1) * TB)
                    for h in range(2):
                        e3 = E[i][:, h * 8192:(h + 1) * 8192].rearrange(
                            "p (b a) -> p b a", a=128)
                        kb = ktile[:, sl].unsqueeze(1).broadcast_to(
                            [128, 64, 128])
                        vb = vtile[:, i * TB + h * 64: i * TB + (h + 1) * 64] \
                            .unsqueeze(2).broadcast_to([128, 64, 128])
                        nc.vector.scalar_tensor_tensor(
                            e3, kb, 1.0, vb, AL.mult, AL.mult)
                    nc.scalar.activation(E[i][:], E[i][:], AF.Exp)

                # ---------- F = suffix-masked column sum of E (local final) --
                FCH = 1024
                Fd = drp.tile([1, FREE], F32, tag="Fd", name="Fd")
                for ch in range(FREE // FCH):
                    cs = slice(ch * FCH, (ch + 1) * FCH)
                    pf = psFp.tile([1, FCH], F32, tag="psF", name="psF")
                    for half in range(2):
                        hs = slice(ch * FCH + half * CH,
                                   ch * FCH + (half + 1) * CH)
                        ps = slice(half * CH, (half + 1) * CH)
                        nc.tensor.matmul(pf[:, ps], mmcast(sufct[:, 0:1]),
                                         mmcast(E[0][:, hs]),
                                         start=True, stop=False)
                        nc.tensor.matmul(pf[:, ps], mmcast(sufct[:, 1:2]),
                                         mmcast(E[1][:, hs]),
                                         start=False, stop=True)
                    fs = gsp.tile([1, FCH], F32, tag="row1k", name="row1k")
                    if ch % 2 == 0:
                        nc.vector.tensor_copy(fs[:], pf[:])
                    else:
                        nc.scalar.copy(fs[:], pf[:])
                    nc.sync.dma_start(Fd[0:1, cs], fs[:])

                # ---------- AllGather F ----------
                AGd = drp.tile([NCORE, FREE], F32, tag="AGd", name="AGd")
                nc.gpsimd.collective_compute(
                    "AllGather", AL.bypass,
                    replica_groups=[list(range(NCORE))],
                    ins=[Fd[:].opt()], outs=[AGd[:].opt()],
                )

                # ---------- G = sum_r gam[r] * F_r  (tile form) ----------
                for r in range(NCORE):
                    nc.sync.dma_start(
                        Fg[:, r * 128:(r + 1) * 128],
                        AGd[r:r + 1, :].rearrange("o (p a) -> (o p) a", a=128))
                nc.vector.memset(Gt[:], 0.0)
                for r in range(NCORE):
                    nc.vector.scalar_tensor_tensor(
                        Gt[:], Fg[:, r * 128:(r + 1) * 128], gamt[:, r:r + 1],
                        Gt[:], AL.mult, AL.add)
                Gd = drp.tile([1, FREE], F32, tag="Gd", name="Gd")
                nc.sync.dma_start(
                    Gd[0:1, :].rearrange("o (p a) -> (o p) a", a=128), Gt[:])

                # ---------- segmented prefix sums (cumsum) + carry inject ----
                # block order: 1 then 0 so nothing conflicts; evac in place.
                for gi in range(FREE // GST):
                    gstage = gsp.tile([1, GST], F32, tag="row1k", name="gstage")
                    nc.sync.dma_start(gstage[:],
                                      Gd[0:1, gi * GST:(gi + 1) * GST])
                    for sub in range(GST // CH):
                        ch = gi * (GST // CH) + sub
                        cs = slice(ch * CH, (ch + 1) * CH)
                        gsl = gstage[0:1, sub * CH:(sub + 1) * CH]
                        for i in (1, 0):
                            pc = psCp.tile([128, CH], F32, tag="psC", name="psC")
                            nc.tensor.matmul(pc[:], mmcast(trit[i][:]),
                                             mmcast(E[i][:, cs]),
                                             start=True, stop=False)
                            if i == 1:
                                nc.tensor.matmul(pc[:], mmcast(crosst[:]),
                                                 mmcast(E[0][:, cs]),
                                                 start=False, stop=False)
                            nc.tensor.matmul(
                                pc[:],
                                mmcast(wrowt[0:1, i * TB:(i + 1) * TB]),
                                mmcast(gsl), start=False, stop=True)
                            # evacuate A chunk back into E's slot (in place)
                            if ch % 2 == 0:
                                nc.vector.tensor_copy(E[i][:, cs], pc[:])
                            else:
                                nc.scalar.copy(E[i][:, cs], pc[:])

                # ---------- S = ln(A + lnbias) in place ----------
                for i in range(NB):
                    nc.scalar.activation(E[i][:], E[i][:], AF.Ln,
                                         bias=lnbt[:, i:i + 1])

                # chunk-final state (only core 7's output is used by host)
                nc.sync.dma_start(hfin_d[l:l + 1, :], E[1][127:128, :])

                # ---------- z[t,b] = sum_a S[t,b,a] * q[t,a]; norm ----------
                BW = QW // 128  # b-columns per sub-block
                for i in range(NB):
                    sl = slice(i * TB, (i + 1) * TB)
                    for qq in range(NQ):
                        s3 = E[i][:, qq * QW:(qq + 1) * QW].rearrange(
                            "p (b a) -> p b a", a=128)
                        qb = qtile[:, sl].unsqueeze(1).broadcast_to(
                            [128, BW, 128])
                        scr = scrp.tile([128, QW], F32, tag="scr", name="scr")
                        scr3 = scr[:].rearrange("p (b a) -> p b a", a=128)
                        nc.gpsimd.tensor_mul(scr3, s3, qb)
                        nc.vector.tensor_reduce(
                            zraw[:, i * TB + qq * BW: i * TB + (qq + 1) * BW],
                            scr3, axis=AX.X, op=AL.add)
                        # norm partial: sum of squares via ACT accumulate
                        scs = scrp.tile([128, QW], F32, tag="scr", name="scr")
                        nc.scalar.activation(scs[:],
                                             E[i][:, qq * QW:(qq + 1) * QW],
                                             AF.Square,
                                             accum_out=nsq[:, i * NQ + qq:
                                                           i * NQ + qq + 1])
                    nc.vector.tensor_reduce(
                        n2[:, i:i + 1],
                        nsq[:, i * NQ:(i + 1) * NQ].unsqueeze(1),
                        axis=AX.X, op=AL.add)
                nc.vector.reciprocal(rn[:], n2[:])
                nc.scalar.activation(rn[:], rn[:], AF.Sqrt)

                # ---------- z scale + transpose ----------
                for i in range(NB):
                    sl = slice(i * TB, (i + 1) * TB)
                    nc.scalar.activation(zraw[:, sl], zraw[:, sl], AF.Copy,
                                         scale=rn[:, i:i + 1])
                    pt = psGp.tile([128, TC], F32, tag="psG", name="psG")
                    nc.tensor.transpose(pt[:, 0:TB], zraw[:, sl], identt[:])
                    nc.vector.tensor_copy(zT[:, sl], pt[:, 0:TB])

                # ---------- ff: lrelu(Wz @ zT + We @ embT + b_ff) ----------
                pff = psGp.tile([128, TC], F32, tag="psG", name="psG")
                nc.tensor.matmul(pff[:], wzTt[l][:], zT[:], start=True,
                                 stop=False)
                nc.tensor.matmul(pff[:], weTt[l][:], embT[:], start=False,
                                 stop=True)
                nc.scalar.activation(lin_out[l][:], pff[:], AF.Lrelu,
                                     bias=bfft[:, l:l + 1], alpha=0.01)

            # ---------- final projection ----------
            po = psGp.tile([128, TC], F32, tag="psG", name="psG")
            nc.tensor.matmul(po[:], woutT[:], lin2T[:], start=True, stop=True)
            nc.scalar.activation(outT[:], po[:], AF.Identity,
                                 bias=boutt[:, 0:1])
            for i in range(NB):
                sl = slice(i * TB, (i + 1) * TB)
                pt = psGp.tile([128, TC], F32, tag="psG", name="psG")
                nc.tensor.transpose(pt[:, 0:TB], outT[:, sl], identt[:])
                nc.vector.tensor_copy(outt[:, sl], pt[:, 0:TB])
                nc.sync.dma_start(out_d[i * TB:(i + 1) * TB, :],
                                  outt[:, sl])

    nc.compile()
    return nc


def _host_fallback(x, start, h0_state, h0_flag, W_in, b_in, W_out, b_out,
                   K, Q, V, gamma, W_ff, b_ff):
    """Numpy implementation (sequential left fold) for inputs outside the
    fast path. Exactly matches the reference for gamma == identity (the
    monoid is then truly associative); for general gamma it is the natural
    sequential semantics of the same recurrence."""
    st = np.asarray(start, bool)
    Tn = x.shape[0]
    emb = x @ W_in.T + b_in
    lin = emb
    h_states, h_flags = [], []
    for l in range(L):
        k = lin @ K[l].T
        q = lin @ Q[l].T
        v = lin @ V[l].T
        S = np.empty((Tn, H, H), np.float32)
        carry = h0_state[l, 0].copy()
        cf = bool(h0_flag[l, 0])
        for t in range(Tn):
            kv = np.outer(k[t], v[t]).astype(np.float32)
            if st[t]:
                s = kv
            else:
                gc = gamma[l] @ carry
                m = np.maximum(gc, kv)
                s = m + np.log1p(np.exp(-np.abs(gc - kv)))
            S[t] = s
            carry = s
            cf = cf or bool(st[t])
        h_states.append(carry.copy()[None])
        h_flags.append(np.array([cf]))
        norm = np.sqrt((S * S).sum(axis=(1, 2)))
        z = np.einsum("ti,tij->tj", q, S) / norm[:, None]
        zc = np.concatenate([z, emb], axis=-1) @ W_ff[l].T + b_ff[l]
        lin = np.where(zc >= 0, zc, 0.01 * zc).astype(np.float32)
    out = lin @ W_out.T + b_out
    return (out.astype(np.float32), np.stack(h_states).astype(np.float32),
            np.stack(h_flags))


def _host_masks(start_np):
    """Per-core 0/1 mask tensors for the segmented scan."""
    s = np.asarray(start_np).astype(bool).astype(np.int64)
    Scs = np.concatenate([[0], np.cumsum(s)])
    t_idx = np.arange(TB)
    per_core = []
    for c in range(NCORE):
        sl = s[c * TC:(c + 1) * TC]
        cs = np.concatenate([[0], np.cumsum(sl)])

        def nof(a, b):
            # no flag among local positions [a, b] inclusive (b < a -> True)
            a = np.asarray(a)
            b = np.asarray(b)
            return (cs[np.minimum(b + 1, TC)] - cs[np.minimum(a, TC)]
                    <= 0) | (b < a)

        tri = np.zeros((NB, TB, TB), np.float32)
        for i in range(NB):
            tt, tp = np.meshgrid(t_idx, t_idx, indexing="ij")
            m = (tt <= tp) & nof(i * TB + tt + 1, i * TB + tp)
            tri[i] = m.astype(np.float32)
        tt, tp = np.meshgrid(t_idx, t_idx, indexing="ij")
        cross = nof(tt + 1, TB + tp).astype(np.float32)
        w = np.zeros((NB, TB), np.float32)
        suf = np.zeros((NB, TB), np.float32)
        for i in range(NB):
            w[i] = nof(np.zeros(TB, np.int64), i * TB + t_idx)
            suf[i] = nof(i * TB + t_idx + 1, np.full(TB, TC - 1))
        gam = np.zeros(NCORE, np.float32)
        for r in range(c):
            gam[r] = 1.0 if (Scs[c * TC] - Scs[(r + 1) * TC]) == 0 else 0.0
        gA0 = 1.0 if Scs[c * TC] == 0 else 0.0
        lnbias = (gA0 * w).astype(np.float32)
        per_core.append(dict(tri=tri, cross=cross, w=w, suf=suf, gam=gam,
                             lnbias=lnbias))
    return per_core


def kernel(x, start, h0_state, h0_flag, W_in, b_in, W_out, b_out,
           K, Q, V, gamma, W_ff, b_ff):
    x = np.asarray(x, np.float32)
    start_np = np.asarray(start).astype(bool)
    h0_state = np.asarray(h0_state, np.float32)
    h0_flag_np = np.asarray(h0_flag).astype(bool)
    W_in = np.asarray(W_in, np.float32)
    b_in = np.asarray(b_in, np.float32)
    W_out = np.asarray(W_out, np.float32)
    b_out = np.asarray(b_out, np.float32)
    K = np.asarray(K, np.float32)
    Q = np.asarray(Q, np.float32)
    V = np.asarray(V, np.float32)
    gamma = np.asarray(gamma, np.float32)
    W_ff = np.asarray(W_ff, np.float32)
    b_ff = np.asarray(b_ff, np.float32)

    eye = np.eye(H, dtype=np.float32)
    fast = (
        x.shape == (T, IN)
        and all(np.allclose(gamma[l], eye) for l in range(L))
        and np.all(h0_state == 0.0)
    )
    if not fast:
        return _host_fallback(x, start_np, h0_state, h0_flag_np, W_in, b_in,
                              W_out, b_out, K, Q, V, gamma, W_ff, b_ff)

    from concourse import bass_utils

    if MM_MODE not in _CACHE:
        _CACHE[MM_MODE] = _build_program(MM_MODE)
    nc = _CACHE[MM_MODE]

    masks = _host_masks(start_np)

    shared = {
        "winT": np.ascontiguousarray(W_in.T),           # [IN, H]
        "woutT": np.ascontiguousarray(W_out.T),         # [H, OUT]
        "kT": np.ascontiguousarray(np.transpose(K, (0, 2, 1))),
        "qT": np.ascontiguousarray(np.transpose(Q, (0, 2, 1))),
        "vT": np.ascontiguousarray(np.transpose(V, (0, 2, 1))),
        "wzT": np.ascontiguousarray(np.transpose(W_ff[:, :, :H], (0, 2, 1))),
        "weT": np.ascontiguousarray(np.transpose(W_ff[:, :, H:], (0, 2, 1))),
        "bin": b_in.reshape(128, 1),
        "bout": b_out.reshape(128, 1),
        "bff": b_ff.reshape(L, 128, 1),
        "ident": np.eye(128, dtype=np.float32),
    }

    in_maps = []
    for c in range(NCORE):
        m = dict(shared)
        m["xT"] = np.ascontiguousarray(x[c * TC:(c + 1) * TC, :].T)
        mk = masks[c]
        m["tri"] = mk["tri"]
        m["cross"] = mk["cross"]
        m["wrow"] = mk["w"].reshape(1, NB * TB)
        m["sufcol"] = np.ascontiguousarray(mk["suf"].T)      # [128, NB]
        m["lnbias"] = np.ascontiguousarray(mk["lnbias"].T)   # [128, NB]
        m["gam"] = np.broadcast_to(mk["gam"], (128, NCORE)).copy()
        in_maps.append(m)

    res = bass_utils.run_bass_kernel_spmd(nc, in_maps, list(range(NCORE)))

    out = np.concatenate([res.results[c]["outc"] for c in range(NCORE)],
                         axis=0)
    hf = res.results[NCORE - 1]["hfin"]       # [L, FREE], row = S[(b,a)]
    h_states = np.stack([hf[l].reshape(H, H).T[None] for l in range(L)])
    any_start = bool(start_np.any())
    h_flags = np.stack([(h0_flag_np[l] | any_start) for l in range(L)])
    return (out.astype(np.float32), h_states.astype(np.float32), h_flags)
